# revision 61
# baseline (speedup 1.0000x reference)
"""CapsuleLayer kernel for Trainium2 (8 NeuronCores, Bass/Tile).

Math: reference einsum("bhwf,fcd->bhwd", x, Wc) sums over BOTH f and c,
so it collapses to a single matmul:
    W_eff[f, d] = sum_c capsules.reshape(F, C, D)[f, c, d]
    out = x.reshape(-1, F) @ W_eff            # (100352, 256) @ (256, 16)

Distribution: data-parallel over flattened positions (batch*H*W), 12544
positions per core; the small capsule weight is replicated. Each core
receives its x shard pre-transposed to (F, PPC) so the contraction dim f
sits on SBUF partitions (the tensor engine contracts over partitions);
the core emits outT (16, PPC) which the host transposes back (6.4 MB).

Modes (host-side dtype of the streamed x shard + PE matmul dtype):
  'fp32' - exact float32 matmul (4 PE cycles/row), full 4-byte stream
  'f32r' - float32r matmul (1 cycle/row), full 4-byte stream
  'fp16' - x/W rounded to fp16 (1 cycle/row), 2-byte stream (half the
           HBM traffic; the kernel is memory-bound so ~2x faster)

Measured (per-core NTFF exec time, 8 cores concurrent):
  fp16 34.5-35.9 us (rel err 2.9e-4), f32r ~52 us (1.5e-4),
  fp32 ~60-67 us (6e-8).
Per-core structure at fp16: ~6 us NEFF/Tile preamble (engine start
rendezvous + IRAM/table fetch), ~16.5 us input stream at fabric line
rate (~395 GB/s) on both HWDGE rings, tail = ~3 us DMA-completion
semaphore lag + col-tiled matmuls (4 position-blocks execute
concurrently in the PE array via tile_position col groups; one
[128,448] DVE copy drains 4 PSUM strips at full lane use) + split
early/late stores, ~4-5 us end drain/barrier.
"""

import numpy as np

import concourse.bass as bass  # noqa: F401  (engine types referenced via nc)
import concourse.tile as tile
from concourse import bacc, mybir
from concourse.bass_utils import run_bass_kernel_spmd

N_CORES = 8
B, H, W, F = 32, 56, 56, 256
NUM_CAPS, CAP_DIM = 10, 16
POS = B * H * W            # 100352
PPC = POS // N_CORES       # 12544 positions per core
SUB = 448                  # matmul moving free dim (<=512 fp32)
NT = 4 * SUB               # 1792 positions per big chunk (4 col-tiled strips)
NBIG = 6                   # 6 big chunks + 4 small tail chunks = 12544
KC = F // 128              # 2 contraction chunks of 128

MODE = "fp16"              # default; see module docstring

_MM_DT = {
    "fp32": mybir.dt.float32,
    "f32r": mybir.dt.float32r,
    "fp16": mybir.dt.float16,
}
_NP_DT = {"fp32": np.float32, "f32r": np.float32, "fp16": np.float16}

_cache = {}


def _build(mode: str):
    nc = bacc.Bacc(
        None,
        target_bir_lowering=False,
        debug=False,
        enable_asserts=False,
        num_devices=N_CORES,
    )
    mm_dt = _MM_DT[mode]

    xT = nc.dram_tensor("xT", [F, PPC], mm_dt, kind="ExternalInput")
    caps = nc.dram_tensor(
        "caps", [F, NUM_CAPS * CAP_DIM], mybir.dt.float32, kind="ExternalInput"
    )
    outT = nc.dram_tensor("outT", [CAP_DIM, PPC], mybir.dt.float32, kind="ExternalOutput")

    with tile.TileContext(nc) as tc:
        with (
            tc.tile_pool(name="const", bufs=1) as cpool,
            tc.tile_pool(name="xinb", bufs=NBIG) as xpool_b,
            tc.tile_pool(name="xins", bufs=4) as xpool_s,
            tc.tile_pool(name="psumb", bufs=4, space="PSUM") as pspool_b,
            tc.tile_pool(name="psums", bufs=4, space="PSUM") as pspool_s,
        ):
            # ---- W_eff = sum over capsules of the (F, C*D) weight --------
            # caps load goes FIRST on the sync ring: weff gates every matmul
            ct = cpool.tile([128, KC, NUM_CAPS * CAP_DIM], mybir.dt.float32, tag="caps")
            nc.sync.dma_start(ct[:], caps.rearrange("(k p) c -> p k c", p=128))
            w32 = cpool.tile([128, KC, CAP_DIM], mybir.dt.float32, tag="w32")
            for k in range(KC):
                # view (128, C*D) as (128, D, C) and reduce the capsule axis
                nc.vector.reduce_sum(
                    w32[:, k, :],
                    ct[:, k, :].rearrange("p (c d) -> p d c", c=NUM_CAPS),
                    axis=mybir.AxisListType.X,
                )
            # single copy writes the whole weff tile before any matmul
            # LDWEIGHTS touches it (concurrent DVE-write/PE-LDW on the same
            # tile was observed to wedge the exec unit in fp16)
            weff = cpool.tile([128, KC, CAP_DIM], mm_dt, tag="weff")
            nc.vector.tensor_copy(weff[:], w32[:])

            # ---- streaming matmul over position chunks -------------------
            # chunk schedule: big chunks for stream efficiency, small ones
            # at the end to shorten the completion-lag tail chain.
            chunks = []
            off = 0
            for sz in [NT] * NBIG + [SUB] * 4:
                chunks.append((off, sz))
                off += sz
            assert off == PPC

            # all chunk tiles resident (shard fits in SBUF): the input DMAs
            # have no buffer-recycle deps, so they queue back-to-back.
            # Chunks alternate between the two HWDGE rings (sync + scalar)
            # so one ring's completion bubble hides under the other.
            xT_v = xT.rearrange("(k p) n -> p k n", k=KC)  # [128, KC, PPC]
            xts = []
            for j, (o, sz) in enumerate(chunks):
                cols = slice(o, o + sz)
                pool = xpool_b if sz == NT else xpool_s
                xt = pool.tile([128, KC, sz], mm_dt, tag=f"xt{sz}")
                ring = nc.sync if j % 2 == 0 else nc.scalar
                ring.dma_start(xt[:], xT_v[:, :, cols])
                xts.append(xt)

            # resident output buffers: column c holds chunk-column c's 4
            # strips. ob_a (cols 0..3) stores early on the SWDGE path while
            # the input stream owns the rings; ob_b (cols 4..5) stores late
            # on the rings; each tail chunk gets its OWN tiny tile so its
            # store fires the moment its copy lands (per-tile deps).
            HALF_A = 3
            ob_a = cpool.tile([128, HALF_A, SUB], mybir.dt.float32, tag="oba")
            ob_b = cpool.tile([128, NBIG - HALF_A, SUB], mybir.dt.float32, tag="obb")
            ob_t = []
            for s in range(4):
                obt = cpool.tile([CAP_DIM, SUB], mybir.dt.float32, tag=f"obt{s}")
                ob_t.append(obt)

            def ob_slot(col):
                if col < HALF_A:
                    return ob_a, col
                return ob_b, col - HALF_A

            for j, (o, sz) in enumerate(chunks):
                xt = xts[j]
                if sz == NT:
                    # 4 col-tiled strips into ONE PSUM bank: sub s lands on
                    # partitions 32s..32s+15, so a single [128, SUB] DVE copy
                    # drains 4 subs at full lane utilization.
                    ps = pspool_b.tile([128, 512], mybir.dt.float32, tag="psb")
                    for s in range(4):
                        sl = slice(s * SUB, (s + 1) * SUB)
                        for k in range(KC):
                            nc.tensor.matmul(
                                ps[32 * s : 32 * s + CAP_DIM, 0:SUB],
                                weff[:, k, :],
                                xt[:, k, sl],
                                start=(k == 0),
                                stop=(k == KC - 1),
                                tile_position=(0, 32 * s),
                            )
                    ob, col = ob_slot(j)
                    nc.vector.tensor_copy(ob[:, col, :], ps[:, 0:SUB])
                else:
                    s = j - NBIG  # strip for this tail chunk
                    ps = pspool_s.tile([CAP_DIM, 512], mybir.dt.float32, tag="pss")
                    for k in range(KC):
                        nc.tensor.matmul(
                            ps[:, 0:SUB],
                            weff[:, k, :],
                            xt[:, k, :],
                            start=(k == 0),
                            stop=(k == KC - 1),
                        )
                    nc.vector.tensor_copy(ob_t[s][:], ps[:, 0:SUB])

            # strip-stores; outT position of (chunk-col c, strip s) = c*NT+s*SUB.
            # ob_a early on SWDGE (rings are busy with input); ob_b late,
            # 2 strips per ring; tail tiles last, each gated only by its
            # own copy, interleaved on both rings.
            outT_s = outT.rearrange("d (c s n) -> d s c n", s=4, n=SUB)
            for s in range(4):
                nc.gpsimd.dma_start(
                    outT_s[:, s, 0:HALF_A, :],
                    ob_a[32 * s : 32 * s + CAP_DIM, :, :],
                )
            for s in range(4):
                ring = nc.sync if s % 2 == 0 else nc.scalar
                ring.dma_start(
                    outT_s[:, s, HALF_A:NBIG, :],
                    ob_b[32 * s : 32 * s + CAP_DIM, :, :],
                )
            for s in range(4):
                ring = nc.sync if s % 2 == 0 else nc.scalar
                ring.dma_start(outT_s[:, s, NBIG, :], ob_t[s][:])

    nc.compile()
    return nc


def _get_nc(mode: str):
    if mode not in _cache:
        _cache[mode] = _build(mode)
    return _cache[mode]


def run(x, capsules, trace=False, trace_cores=None, mode=None):
    """Shard, execute on 8 cores, gather. Returns (out, BassKernelResults)."""
    if mode is None:
        mode = MODE
    nc = _get_nc(mode)

    x = np.asarray(x, dtype=np.float32)
    capsules = np.asarray(capsules, dtype=np.float32)
    xf = x.reshape(POS, F).astype(_NP_DT[mode], copy=False)
    caps2 = np.ascontiguousarray(capsules.reshape(F, NUM_CAPS * CAP_DIM))
    xT_full = xf.T  # view; per-core slices are copied once during input concat

    in_maps = [
        {"xT": xT_full[:, c * PPC : (c + 1) * PPC], "caps": caps2}
        for c in range(N_CORES)
    ]
    res = run_bass_kernel_spmd(
        nc,
        in_maps,
        core_ids=list(range(N_CORES)),
        trace=trace,
        trace_cores=trace_cores,
    )
    out = np.empty((POS, CAP_DIM), dtype=np.float32)
    for c in range(N_CORES):
        out[c * PPC : (c + 1) * PPC] = res.results[c]["outT"].T
    return out.reshape(B, H, W, CAP_DIM), res


def kernel(x, capsules):
    out, _ = run(x, capsules)
    return out


# revision 62
# speedup vs baseline: 1.0314x; 1.0314x over previous
"""CapsuleLayer kernel for Trainium2 (8 NeuronCores, Bass/Tile).

Math: reference einsum("bhwf,fcd->bhwd", x, Wc) sums over BOTH f and c,
so it collapses to a single matmul:
    W_eff[f, d] = sum_c capsules.reshape(F, C, D)[f, c, d]
    out = x.reshape(-1, F) @ W_eff            # (100352, 256) @ (256, 16)

Distribution: data-parallel over flattened positions (batch*H*W), 12544
positions per core; the small capsule weight is replicated. Each core
receives its x shard pre-transposed to (F, PPC) so the contraction dim f
sits on SBUF partitions (the tensor engine contracts over partitions);
the core emits outT (16, PPC) which the host transposes back (6.4 MB).

Modes (host-side dtype of the streamed x shard + PE matmul dtype):
  'fp32' - exact float32 matmul (4 PE cycles/row), full 4-byte stream
  'f32r' - float32r matmul (1 cycle/row), full 4-byte stream
  'fp16' - x/W rounded to fp16 (1 cycle/row), 2-byte stream (half the
           HBM traffic; the kernel is memory-bound so ~2x faster)

Measured (per-core NTFF exec time, 8 cores concurrent):
  fp16 34.5-35.9 us (rel err 2.9e-4), f32r ~52 us (1.5e-4),
  fp32 ~60-67 us (6e-8).
Per-core structure at fp16: ~6 us NEFF/Tile preamble (engine start
rendezvous + IRAM/table fetch), ~16.5 us input stream at fabric line
rate (~395 GB/s) on both HWDGE rings, tail = ~3 us DMA-completion
semaphore lag + col-tiled matmuls (4 position-blocks execute
concurrently in the PE array via tile_position col groups; one
[128,448] DVE copy drains 4 PSUM strips at full lane use) + split
early/late stores, ~4-5 us end drain/barrier.
"""

import numpy as np

import concourse.bass as bass  # noqa: F401  (engine types referenced via nc)
import concourse.tile as tile
from concourse import bacc, mybir
from concourse.bass_utils import run_bass_kernel_spmd

N_CORES = 8
B, H, W, F = 32, 56, 56, 256
NUM_CAPS, CAP_DIM = 10, 16
POS = B * H * W            # 100352
PPC = POS // N_CORES       # 12544 positions per core
SUB = 448                  # matmul moving free dim (<=512 fp32)
NT = 4 * SUB               # 1792 positions per big chunk (4 col-tiled strips)
NBIG = 6                   # 6 big chunks + 4 small tail chunks = 12544
KC = F // 128              # 2 contraction chunks of 128

MODE = "fp16"              # default; see module docstring

_MM_DT = {
    "fp32": mybir.dt.float32,
    "f32r": mybir.dt.float32r,
    "fp16": mybir.dt.float16,
}
_NP_DT = {"fp32": np.float32, "f32r": np.float32, "fp16": np.float16}

_cache = {}


def _build(mode: str):
    nc = bacc.Bacc(
        None,
        target_bir_lowering=False,
        debug=False,
        enable_asserts=False,
        num_devices=N_CORES,
    )
    mm_dt = _MM_DT[mode]

    xT = nc.dram_tensor("xT", [F, PPC], mm_dt, kind="ExternalInput")
    caps = nc.dram_tensor(
        "caps", [F, NUM_CAPS * CAP_DIM], mybir.dt.float32, kind="ExternalInput"
    )
    outT = nc.dram_tensor("outT", [CAP_DIM, PPC], mybir.dt.float32, kind="ExternalOutput")

    with tile.TileContext(nc) as tc:
        with (
            tc.tile_pool(name="const", bufs=1) as cpool,
            tc.tile_pool(name="xinb", bufs=NBIG) as xpool_b,
            tc.tile_pool(name="xins", bufs=4) as xpool_s,
            tc.tile_pool(name="psumb", bufs=4, space="PSUM") as pspool_b,
            tc.tile_pool(name="psums", bufs=4, space="PSUM") as pspool_s,
        ):
            # ---- W_eff = sum over capsules of the (F, C*D) weight --------
            # caps load goes FIRST on the sync ring: weff gates every matmul
            ct = cpool.tile([128, KC, NUM_CAPS * CAP_DIM], mybir.dt.float32, tag="caps")
            nc.sync.dma_start(ct[:], caps.rearrange("(k p) c -> p k c", p=128))
            w32 = cpool.tile([128, KC, CAP_DIM], mybir.dt.float32, tag="w32")
            for k in range(KC):
                # view (128, C*D) as (128, D, C) and reduce the capsule axis
                nc.vector.reduce_sum(
                    w32[:, k, :],
                    ct[:, k, :].rearrange("p (c d) -> p d c", c=NUM_CAPS),
                    axis=mybir.AxisListType.X,
                )
            # single copy writes the whole weff tile before any matmul
            # LDWEIGHTS touches it (concurrent DVE-write/PE-LDW on the same
            # tile was observed to wedge the exec unit in fp16)
            weff = cpool.tile([128, KC, CAP_DIM], mm_dt, tag="weff")
            nc.vector.tensor_copy(weff[:], w32[:])

            # ---- streaming matmul over position chunks -------------------
            # chunk schedule: big chunks for stream efficiency, small ones
            # at the end to shorten the completion-lag tail chain.
            chunks = []
            off = 0
            for sz in [NT] * NBIG + [SUB] * 4:
                chunks.append((off, sz))
                off += sz
            assert off == PPC

            # all chunk tiles resident (shard fits in SBUF): the input DMAs
            # have no buffer-recycle deps, so they queue back-to-back.
            # Chunks alternate between the two HWDGE rings (sync + scalar)
            # so one ring's completion bubble hides under the other.
            xT_v = xT.rearrange("(k p) n -> p k n", k=KC)  # [128, KC, PPC]
            xts = []
            for j, (o, sz) in enumerate(chunks):
                cols = slice(o, o + sz)
                pool = xpool_b if sz == NT else xpool_s
                xt = pool.tile([128, KC, sz], mm_dt, tag=f"xt{sz}")
                ring = nc.sync if j % 2 == 0 else nc.scalar
                ring.dma_start(xt[:], xT_v[:, :, cols])
                xts.append(xt)

            # resident output buffers: column c holds chunk-column c's 4
            # strips. ob_a (cols 0..3) stores early on the SWDGE path while
            # the input stream owns the rings; ob_b (cols 4..5) stores late
            # on the rings; each tail chunk gets its OWN tiny tile so its
            # store fires the moment its copy lands (per-tile deps).
            HALF_A = 4
            ob_a = cpool.tile([128, HALF_A, SUB], mybir.dt.float32, tag="oba")
            ob_b = cpool.tile([128, NBIG - HALF_A, SUB], mybir.dt.float32, tag="obb")
            ob_t = []
            for s in range(4):
                obt = cpool.tile([CAP_DIM, SUB], mybir.dt.float32, tag=f"obt{s}")
                ob_t.append(obt)

            def ob_slot(col):
                if col < HALF_A:
                    return ob_a, col
                return ob_b, col - HALF_A

            for j, (o, sz) in enumerate(chunks):
                xt = xts[j]
                if sz == NT:
                    # 4 col-tiled strips into ONE PSUM bank: sub s lands on
                    # partitions 32s..32s+15, so a single [128, SUB] DVE copy
                    # drains 4 subs at full lane utilization.
                    ps = pspool_b.tile([128, 512], mybir.dt.float32, tag="psb")
                    for s in range(4):
                        sl = slice(s * SUB, (s + 1) * SUB)
                        for k in range(KC):
                            nc.tensor.matmul(
                                ps[32 * s : 32 * s + CAP_DIM, 0:SUB],
                                weff[:, k, :],
                                xt[:, k, sl],
                                start=(k == 0),
                                stop=(k == KC - 1),
                                tile_position=(0, 32 * s),
                            )
                    ob, col = ob_slot(j)
                    nc.vector.tensor_copy(ob[:, col, :], ps[:, 0:SUB])
                else:
                    s = j - NBIG  # strip for this tail chunk
                    ps = pspool_s.tile([CAP_DIM, 512], mybir.dt.float32, tag="pss")
                    for k in range(KC):
                        nc.tensor.matmul(
                            ps[:, 0:SUB],
                            weff[:, k, :],
                            xt[:, k, :],
                            start=(k == 0),
                            stop=(k == KC - 1),
                        )
                    nc.vector.tensor_copy(ob_t[s][:], ps[:, 0:SUB])

            # strip-stores; outT position of (chunk-col c, strip s) = c*NT+s*SUB.
            # ob_a early on SWDGE (rings are busy with input); ob_b late,
            # 2 strips per ring; tail tiles last, each gated only by its
            # own copy, interleaved on both rings.
            outT_s = outT.rearrange("d (c s n) -> d s c n", s=4, n=SUB)
            for s in range(4):
                nc.gpsimd.dma_start(
                    outT_s[:, s, 0:HALF_A, :],
                    ob_a[32 * s : 32 * s + CAP_DIM, :, :],
                )
            for s in range(4):
                ring = nc.sync if s % 2 == 0 else nc.scalar
                ring.dma_start(
                    outT_s[:, s, HALF_A:NBIG, :],
                    ob_b[32 * s : 32 * s + CAP_DIM, :, :],
                )
            for s in range(4):
                ring = nc.sync if s % 2 == 0 else nc.scalar
                ring.dma_start(outT_s[:, s, NBIG, :], ob_t[s][:])

    nc.compile()
    return nc


def _get_nc(mode: str):
    if mode not in _cache:
        _cache[mode] = _build(mode)
    return _cache[mode]


def run(x, capsules, trace=False, trace_cores=None, mode=None):
    """Shard, execute on 8 cores, gather. Returns (out, BassKernelResults)."""
    if mode is None:
        mode = MODE
    nc = _get_nc(mode)

    x = np.asarray(x, dtype=np.float32)
    capsules = np.asarray(capsules, dtype=np.float32)
    xf = x.reshape(POS, F).astype(_NP_DT[mode], copy=False)
    caps2 = np.ascontiguousarray(capsules.reshape(F, NUM_CAPS * CAP_DIM))
    xT_full = xf.T  # view; per-core slices are copied once during input concat

    in_maps = [
        {"xT": xT_full[:, c * PPC : (c + 1) * PPC], "caps": caps2}
        for c in range(N_CORES)
    ]
    res = run_bass_kernel_spmd(
        nc,
        in_maps,
        core_ids=list(range(N_CORES)),
        trace=trace,
        trace_cores=trace_cores,
    )
    out = np.empty((POS, CAP_DIM), dtype=np.float32)
    for c in range(N_CORES):
        out[c * PPC : (c + 1) * PPC] = res.results[c]["outT"].T
    return out.reshape(B, H, W, CAP_DIM), res


def kernel(x, capsules):
    out, _ = run(x, capsules)
    return out
